# revision 29
# baseline (speedup 1.0000x reference)
"""GAT (2-layer, 4-head) Trainium2 Bass kernel, sharded across 8 NeuronCores.

Sharding: 1D row partition of the dense NxN attention; each core owns 1024
rows (queries). Full h = x @ W computed locally, row-block of masked softmax
attention and att @ h per layer, xcat all-gathered between layers.

v2 core trick — fused score+exp on the DVE via a custom 5-stage op:
    K = max(y, 0.2*y) + 16256,  y = (f1bc128 + f2_128[j]) + mask128
computed on scores pre-scaled by 128*log2e (folded into the weights on
host), written to an int16 tile. fp32->int16 write-conversion makes the
VALUE the bf16 BIT PATTERN: reinterpreting the int16 bytes as bf16 yields
2^lrelu(z)' (Schraudolph linear-interp exp2, |err|<=6%, cancels in softmax).
The att @ h matmul reads the tile via .bitcast(bf16) directly - one DVE
instruction per (head, j-tile), no ACT exp, no separate mask op.

Remaining units run an ACT path (gpsimd tensor-tensor add -> Prelu(bias=f2)
-> Exp(scale=ln2/128)) to balance DVE vs ACT vs GpSimd load. adj reaches the
device only as a host-precomputed bf16 additive mask {0, -229376}.
"""

import os
import sys
from contextlib import ExitStack

import numpy as np

sys.path.insert(0, "/opt/trn_rl_repo")

import ml_dtypes

import concourse.bass as bass
import concourse.tile as tile
from concourse import bacc, mybir
from concourse.bass_utils import run_bass_kernel_spmd

from concourse.dve_ops import (
    DveOp,
    OPS,
    CUSTOM_DVE_SPECS,
    _SUB_OPCODE_FOR_NAME,
    _CUSTOM_DVE_ROW_BASE,
)
from concourse.dve_spec import Spec, Src0, Src1, C0, C1, C2, maxx, lower, _has_src1
from concourse.dve_uop import DveOpSpec

BF16 = ml_dtypes.bfloat16
F32 = mybir.dt.float32
BF = mybir.dt.bfloat16
I16 = mybir.dt.int16

N, NFEAT, NHID, NCLASS, NHEADS, NCORES = 8192, 512, 64, 16, 4, 8
ROWS = N // NCORES          # 1024 rows per core
JT = N // 128               # 64 j-tiles
IT = ROWS // 128            # 8 i-tiles
KT1 = NFEAT // 128          # 4 k-tiles layer-1
FCAT = NHEADS * NHID        # 256
KT2 = FCAT // 128           # 2 k-tiles layer-2
ALPHA = 0.2
OUT_SLOPE = 0.01

SCL = 128.0 * 1.4426950408889634   # fold into f1/f2: scores in 128*log2 domain
MASKV = -229376.0                  # additive mask, bf16-exact (-1.75*2^17)
BIAS = 16256.0 - 7.364             # (127<<7) minus Schraudolph mean-centering
LN2_128 = float(np.log(2.0) / 128.0)

AluOp = mybir.AluOpType
ActFn = mybir.ActivationFunctionType

# --- path config (tunable): which (unit) runs the DVE fused path vs ACT ----
def L1_IS_DVE(k, jt):           # layer-1 (head, jt) on the fused DVE path?
    if k <= 1:
        return True
    return (jt // 2) % 8 in (0, 3, 6)   # head 2 (head 3 runs inside phase A)
def L2_IS_DVE(jt):              # layer-2: pair-aligned, 2/3 on DVE
    return (jt // 2) % 3 != 2


def _register_gat_exp():
    """Fused masked-score -> Schraudolph-exp2 DVE op (int16 output)."""
    name = "GAT_EXP5B"
    if name in CUSTOM_DVE_SPECS:
        return next(op for op in OPS if op.name == name)

    def ref(in0, in1, s0, s1, imm2):
        y = in0.astype(np.float32) + np.float32(s0) + in1.astype(np.float32)
        l = np.maximum(y, np.float32(s1) * y)
        K = l + np.float32(imm2)
        return np.clip(np.rint(K), -32768, 32767).astype(np.int16)

    _y = (Src0 + C0) + Src1
    spec = Spec(body=maxx(_y, _y * C1) + C2, reference=ref)
    row = _CUSTOM_DVE_ROW_BASE + len(OPS)
    assert row < 0x20
    uops = lower(spec, ver="v3")
    sha = DveOpSpec(name=name, opcode=row, uops=uops, rd1_en=_has_src1(spec)).sha("v3")
    op = DveOp(name=name, spec=spec, subdim=False, uops_sha={"v3": sha})
    OPS.append(op)
    CUSTOM_DVE_SPECS[name] = spec
    _SUB_OPCODE_FOR_NAME[name] = row
    return op


GAT_EXP = _register_gat_exp()


def _compile_with_single_act_table(nc):
    """Restrict activations to two HW table sets: the Exp set (phases B/D)
    and the Reciprocal set (used once, in the phase-B epilogue)."""
    import concourse.bacc as bacc_mod

    orig = bacc_mod.get_activation_tables
    need = {
        mybir.ActivationFunctionType.Exp,
        mybir.ActivationFunctionType.Prelu,
        mybir.ActivationFunctionType.Copy,
        mybir.ActivationFunctionType.Identity,
    }
    need_r = {
        mybir.ActivationFunctionType.Reciprocal,
        mybir.ActivationFunctionType.Prelu,
        mybir.ActivationFunctionType.Copy,
    }

    def restricted(arch):
        tables = orig(arch)
        out = {}
        for k, v in tables.items():
            if need <= set(v):
                out[k] = v
                break
        for k, v in tables.items():
            if need_r <= set(v):
                out[k] = v
                break
        return out or tables

    bacc_mod.get_activation_tables = restricted
    try:
        nc.compile()
    finally:
        bacc_mod.get_activation_tables = orig


def build_nc():
    nc = bacc.Bacc(
        "TRN2", target_bir_lowering=False, debug=False, num_devices=NCORES
    )

    # ---- I/O -------------------------------------------------------------
    xT_d = nc.dram_tensor("xT", [NFEAT, N], BF, kind="ExternalInput")
    xTown_d = nc.dram_tensor("xTown", [NFEAT, ROWS], BF, kind="ExternalInput")
    maskT_d = nc.dram_tensor("maskT", [N, ROWS], BF, kind="ExternalInput")
    wcat_d = nc.dram_tensor("wcat", [NFEAT, FCAT + NHEADS], BF, kind="ExternalInput")
    wa1bc_d = nc.dram_tensor("wa1bc", [NFEAT, NHEADS, 128], BF, kind="ExternalInput")
    woext_d = nc.dram_tensor("woext", [FCAT, NCLASS + 1], BF, kind="ExternalInput")
    woa1bc_d = nc.dram_tensor("woa1bc", [FCAT, 128], BF, kind="ExternalInput")
    out_d = nc.dram_tensor("out", [ROWS, NCLASS], F32, kind="ExternalOutput")
    # layer-2 gather: own-rows h2 (16 cols bf16) + f2*SCL (1 col f32 as 2
    # bf16 cols) + pad, gathered across cores
    g_d = nc.dram_tensor("g_bounce", [ROWS, 20], BF, kind="Internal")
    gg_d = nc.dram_tensor(
        "gg_bounce", [N, 20], BF, kind="Internal", addr_space="Shared"
    )

    dma = nc.default_dma_engine

    with tile.TileContext(nc) as tc, ExitStack() as ctx:
        persist = ctx.enter_context(tc.tile_pool(name="persist", bufs=1))

        h_all = persist.tile([128, JT, NHEADS, NHID + 1], BF)
        fstore = persist.tile([128, JT, NHEADS], F32)      # f2 * SCL
        f1bc = persist.tile([128, NHEADS, ROWS], BF)       # f1 * SCL bcast
        xcT_sb = persist.tile([128, KT2, ROWS], BF)
        h2_all = persist.tile([128, JT, NCLASS + 1], BF)
        fstore2 = persist.tile([128, JT], F32)
        f1bc2 = persist.tile([128, ROWS], BF)
        out_sb = persist.tile([128, IT, NCLASS], F32)

        nc.gpsimd.memset(h_all[:, :, :, NHID : NHID + 1], 1.0)
        nc.gpsimd.memset(h2_all[:, :, NCLASS : NCLASS + 1], 1.0)

        # head-3 accumulator lives across phase A (its ACT-path score work
        # fills the otherwise idle Scalar engine during the h matmuls)
        p3_ps = ctx.enter_context(tc.tile_pool(name="p3_ps", bufs=1, space="PSUM"))
        oT3 = p3_ps.tile([NHID + 1, ROWS], F32, name="oT3")
        p3_m = ctx.enter_context(tc.tile_pool(name="p3_m", bufs=2))
        p3_z = ctx.enter_context(tc.tile_pool(name="p3_z", bufs=2))
        p3_zl = ctx.enter_context(tc.tile_pool(name="p3_zl", bufs=2))
        p3_s = ctx.enter_context(tc.tile_pool(name="p3_s", bufs=2))

        # ================= Phase A: h + f1/f2 =============================
        with ExitStack() as actx:
            pa = actx.enter_context(tc.tile_pool(name="pa", bufs=1))
            pa_ps = actx.enter_context(
                tc.tile_pool(name="pa_ps", bufs=2, space="PSUM")
            )

            xT_sb = pa.tile([128, KT1, N], BF)
            for kt in range(KT1):
                dma.dma_start(
                    out=xT_sb[:, kt, :],
                    in_=xT_d[kt * 128 : (kt + 1) * 128, :],
                )
            xTown_sb = pa.tile([128, KT1, ROWS], BF)
            dma.dma_start(
                out=xTown_sb[:],
                in_=xTown_d[:, :].rearrange("(kt p) f -> p kt f", p=128),
            )
            wcat_sb = pa.tile([128, KT1, FCAT + NHEADS], BF)
            dma.dma_start(
                out=wcat_sb[:],
                in_=wcat_d[:, :].rearrange("(kt p) c -> p kt c", p=128),
            )
            wa1bc_sb = pa.tile([128, KT1, NHEADS, 128], BF)
            dma.dma_start(
                out=wa1bc_sb[:],
                in_=wa1bc_d[:, :, :].rearrange("(kt p) h m -> p kt h m", p=128),
            )

            # f1 broadcast for head 3 first (it feeds the fused head-3 path)
            for k in (3, 0, 1, 2):
                f1p = pa_ps.tile([128, ROWS], F32, tag="f1p")
                for kt in range(KT1):
                    for c in range(ROWS // 512):
                        nc.tensor.matmul(
                            f1p[:, c * 512 : (c + 1) * 512],
                            lhsT=wa1bc_sb[:, kt, k, :],
                            rhs=xTown_sb[:, kt, c * 512 : (c + 1) * 512],
                            start=(kt == 0),
                            stop=(kt == KT1 - 1),
                        )
                nc.vector.tensor_copy(out=f1bc[:, k, :], in_=f1p[:])

            for jt in range(JT):
                hp = pa_ps.tile([128, FCAT + NHEADS], F32, tag="hp")
                for kt in range(KT1):
                    nc.tensor.matmul(
                        hp[:],
                        lhsT=xT_sb[:, kt, jt * 128 : (jt + 1) * 128],
                        rhs=wcat_sb[:, kt, :],
                        start=(kt == 0),
                        stop=(kt == KT1 - 1),
                    )
                # h copy on ACT engine, f2 copy on DVE (small)
                nc.scalar.copy(
                    out=h_all[:, jt, :, 0:NHID],
                    in_=hp[:, 0:FCAT].rearrange("p (h d) -> p h d", h=NHEADS),
                )
                nc.vector.tensor_copy(
                    out=fstore[:, jt, :], in_=hp[:, FCAT : FCAT + NHEADS]
                )
                if jt % 2 == 1:
                    # head-3 ACT path for the completed pair (jt-1, jt)
                    jt2 = jt // 2
                    mt3 = p3_m.tile([128, 2, ROWS], BF, tag="mt3")
                    dma.dma_start(
                        out=mt3[:],
                        in_=maskT_d[jt2 * 256 : (jt2 + 1) * 256, :].rearrange(
                            "(t p) i -> p t i", p=128
                        ),
                    )
                    f1k = f1bc[:, 3, :]
                    f1_bc3 = bass.AP(
                        tensor=f1k.tensor,
                        offset=f1k.offset,
                        ap=[f1k.ap[0], [0, 2], f1k.ap[1]],
                    )
                    zt3 = p3_z.tile([128, 2, ROWS], BF, tag="zt3")
                    nc.vector.tensor_tensor(
                        out=zt3[:].rearrange("p t r -> p (t r)"),
                        in0=f1_bc3,
                        in1=mt3[:].rearrange("p t r -> p (t r)"),
                        op=AluOp.add,
                    )
                    zl3 = p3_zl.tile([128, 2, ROWS], BF, tag="zl3")
                    for t in range(2):
                        jtp = jt2 * 2 + t
                        nc.scalar.activation(
                            out=zl3[:, t, :],
                            in_=zt3[:, t, :],
                            func=ActFn.Prelu,
                            bias=fstore[:, jtp, 3:4],
                            scale=1.0,
                            alpha=ALPHA,
                        )
                    st3 = p3_s.tile([128, 2, ROWS], BF, tag="st3")
                    nc.scalar.activation(
                        out=st3[:].rearrange("p t r -> p (t r)"),
                        in_=zl3[:].rearrange("p t r -> p (t r)"),
                        func=ActFn.Exp,
                        bias=0.0,
                        scale=LN2_128,
                    )
                    for t in range(2):
                        jtp = jt2 * 2 + t
                        for c in range(ROWS // 512):
                            nc.tensor.matmul(
                                oT3[:, c * 512 : (c + 1) * 512],
                                lhsT=h_all[:, jtp, 3, :],
                                rhs=st3[:, t, c * 512 : (c + 1) * 512],
                                start=(jtp == 0),
                                stop=(jtp == JT - 1),
                            )

        # ================= Phase B: layer-1 attention =====================
        pe_sb = ctx.enter_context(tc.tile_pool(name="pe_sb", bufs=1))
        with ExitStack() as bctx:
            pb_m = bctx.enter_context(tc.tile_pool(name="pb_m", bufs=3))
            pb_k = bctx.enter_context(tc.tile_pool(name="pb_k", bufs=4))
            pb_z = bctx.enter_context(tc.tile_pool(name="pb_z", bufs=2))
            pb_zl = bctx.enter_context(tc.tile_pool(name="pb_zl", bufs=2))
            pb_s = bctx.enter_context(tc.tile_pool(name="pb_s", bufs=2))
            pb_ps = bctx.enter_context(
                tc.tile_pool(name="pb_ps", bufs=1, space="PSUM")
            )

            oT = [
                pb_ps.tile([NHID + 1, ROWS], F32, tag=f"oT{k}", name=f"oT{k}")
                for k in range(NHEADS - 1)
            ] + [oT3]

            for jt2 in range(JT // 2):
                mt = pb_m.tile([128, 2, ROWS], BF, tag="mt")
                dma.dma_start(
                    out=mt[:],
                    in_=maskT_d[jt2 * 256 : (jt2 + 1) * 256, :].rearrange(
                        "(t p) i -> p t i", p=128
                    ),
                )
                for k in range(NHEADS - 1):
                    if L1_IS_DVE(k, jt2 * 2):
                        # fused DVE path (per jt)
                        for t in range(2):
                            jt = jt2 * 2 + t
                            kt16 = pb_k.tile([128, ROWS], I16, tag=f"k16_{k}{t}")
                            nc.vector._custom_dve(
                                GAT_EXP,
                                out=kt16[:],
                                in0=f1bc[:, k, :],
                                in1=mt[:, t, :],
                                s0=fstore[:, jt, k : k + 1],
                                s1=ALPHA,
                                imm2=BIAS,
                            )
                            for c in range(ROWS // 512):
                                nc.tensor.matmul(
                                    oT[k][:, c * 512 : (c + 1) * 512],
                                    lhsT=h_all[:, jt, k, :],
                                    rhs=kt16[:, c * 512 : (c + 1) * 512].bitcast(BF),
                                    start=(jt == 0),
                                    stop=(jt == JT - 1),
                                )
                    else:
                        # ACT path, pair-batched TT on DVE
                        f1k = f1bc[:, k, :]
                        f1_bc2 = bass.AP(
                            tensor=f1k.tensor,
                            offset=f1k.offset,
                            ap=[f1k.ap[0], [0, 2], f1k.ap[1]],
                        )
                        zt = pb_z.tile([128, 2, ROWS], BF, tag=f"zt{k}")
                        nc.vector.tensor_tensor(
                            out=zt[:].rearrange("p t r -> p (t r)"),
                            in0=f1_bc2,
                            in1=mt[:].rearrange("p t r -> p (t r)"),
                            op=AluOp.add,
                        )
                        zl = pb_zl.tile([128, 2, ROWS], BF, tag=f"zl{k}")
                        for t in range(2):
                            jt = jt2 * 2 + t
                            nc.scalar.activation(
                                out=zl[:, t, :],
                                in_=zt[:, t, :],
                                func=ActFn.Prelu,
                                bias=fstore[:, jt, k : k + 1],
                                scale=1.0,
                                alpha=ALPHA,
                            )
                        st = pb_s.tile([128, 2, ROWS], BF, tag=f"st{k}")
                        nc.scalar.activation(
                            out=st[:].rearrange("p t r -> p (t r)"),
                            in_=zl[:].rearrange("p t r -> p (t r)"),
                            func=ActFn.Exp,
                            bias=0.0,
                            scale=LN2_128,
                        )
                        for t in range(2):
                            jt = jt2 * 2 + t
                            for c in range(ROWS // 512):
                                nc.tensor.matmul(
                                    oT[k][:, c * 512 : (c + 1) * 512],
                                    lhsT=h_all[:, jt, k, :],
                                    rhs=st[:, t, c * 512 : (c + 1) * 512],
                                    start=(jt == 0),
                                    stop=(jt == JT - 1),
                                )

            osb = [
                pe_sb.tile([NHID + 1, ROWS], F32, tag=f"osb{k}", name=f"osb{k}")
                for k in range(NHEADS)
            ]
            for k in range(NHEADS):
                if k % 2 == 0:
                    nc.vector.tensor_copy(out=osb[k][:], in_=oT[k][:])
                else:
                    nc.scalar.copy(out=osb[k][:], in_=oT[k][:])

        # weights for layer 2 (independent of the collective; load early)
        pw = ctx.enter_context(tc.tile_pool(name="pw", bufs=1))
        woext_sb = pw.tile([128, KT2, NCLASS + 1], BF)
        dma.dma_start(
            out=woext_sb[:],
            in_=woext_d[:, :].rearrange("(kt p) c -> p kt c", p=128),
        )
        woa1bc_sb = pw.tile([128, KT2, 128], BF)
        dma.dma_start(
            out=woa1bc_sb[:],
            in_=woa1bc_d[:, :].rearrange("(kt p) m -> p kt m", p=128),
        )

        # epilogue: normalize + out-lrelu + pack xcatT
        with ExitStack() as ectx:
            pe_ps = ectx.enter_context(
                tc.tile_pool(name="pe_ps", bufs=2, space="PSUM")
            )
            pe_u = ectx.enter_context(tc.tile_pool(name="pe_u", bufs=2))
            ones_sb = ectx.enter_context(tc.tile_pool(name="ones", bufs=1)).tile(
                [1, NHID], F32
            )
            nc.gpsimd.memset(ones_sb[:], 1.0)
            rrow = ectx.enter_context(tc.tile_pool(name="rrow", bufs=1))

            # gather the 4 denominator rows onto partitions 0-3, one DVE
            # reciprocal (cost = free size), scatter back to partition 0
            dd = rrow.tile([4, ROWS], F32)
            for k in range(NHEADS):
                dma.dma_start(out=dd[k : k + 1, :], in_=osb[k][NHID : NHID + 1, :])
            rr = rrow.tile([4, ROWS], F32)
            nc.vector.reciprocal(out=rr[:], in_=dd[:])
            rs4 = [rrow.tile([1, ROWS], F32, name=f"rs{k}") for k in range(NHEADS)]
            for k in range(NHEADS):
                dma.dma_start(out=rs4[k][:], in_=rr[k : k + 1, :])

            for k in range(NHEADS):
                rs = rs4[k]
                rbc = pe_ps.tile([NHID, ROWS], F32, tag="rbc")
                for c in range(ROWS // 512):
                    nc.tensor.matmul(
                        rbc[:, c * 512 : (c + 1) * 512],
                        lhsT=ones_sb[:],
                        rhs=rs[:, c * 512 : (c + 1) * 512],
                        start=True,
                        stop=True,
                    )
                u = pe_u.tile([NHID, ROWS], F32, tag="u")
                nc.vector.tensor_tensor(
                    out=u[:], in0=osb[k][0:NHID, :], in1=rbc[:], op=AluOp.mult
                )
                nc.vector.scalar_tensor_tensor(
                    out=xcT_sb[(k % 2) * NHID : (k % 2) * NHID + NHID, k // 2, :],
                    in0=u[:],
                    scalar=OUT_SLOPE,
                    in1=u[:],
                    op0=AluOp.mult,
                    op1=AluOp.max,
                )

        # ============ Phase C: local h2/f2 for own rows, small all-gather ==
        with ExitStack() as cctx:
            pc = cctx.enter_context(tc.tile_pool(name="pc", bufs=1))
            pc_ps = cctx.enter_context(
                tc.tile_pool(name="pc_ps", bufs=2, space="PSUM")
            )

            h2own = pc.tile([128, IT, NCLASS], BF)
            f2own = pc.tile([128, IT, 1], F32)
            for it in range(IT):
                h2p = pc_ps.tile([128, NCLASS + 1], F32, tag="h2p")
                for kt in range(KT2):
                    nc.tensor.matmul(
                        h2p[:],
                        lhsT=xcT_sb[:, kt, it * 128 : (it + 1) * 128],
                        rhs=woext_sb[:, kt, :],
                        start=(kt == 0),
                        stop=(kt == KT2 - 1),
                    )
                nc.vector.tensor_copy(
                    out=h2own[:, it, :], in_=h2p[:, 0:NCLASS]
                )
                nc.vector.tensor_copy(
                    out=f2own[:, it, :], in_=h2p[:, NCLASS : NCLASS + 1]
                )
            dma.dma_start(
                out=g_d[:, 0:NCLASS].rearrange("(it p) c -> p it c", p=128),
                in_=h2own[:],
            )
            dma.dma_start(
                out=g_d[:, NCLASS : NCLASS + 2]
                .bitcast(F32)
                .rearrange("(it p) c -> p it c", p=128),
                in_=f2own[:],
            )
            nc.gpsimd.collective_compute(
                "AllGather",
                AluOp.bypass,
                replica_groups=[list(range(NCORES))],
                ins=[g_d[:, :].opt()],
                outs=[gg_d[:, :].opt()],
            )
            dma.dma_start(
                out=h2_all[:, :, 0:NCLASS],
                in_=gg_d[:, 0:NCLASS].rearrange("(jt p) c -> p jt c", p=128),
            )
            dma.dma_start(
                out=fstore2[:, :],
                in_=gg_d[:, NCLASS : NCLASS + 2]
                .bitcast(F32)
                .rearrange("(jt p) c -> p (jt c)", p=128),
            )

            f1p2 = pc_ps.tile([128, ROWS], F32, tag="f1p2")
            for kt in range(KT2):
                for c in range(ROWS // 512):
                    nc.tensor.matmul(
                        f1p2[:, c * 512 : (c + 1) * 512],
                        lhsT=woa1bc_sb[:, kt, :],
                        rhs=xcT_sb[:, kt, c * 512 : (c + 1) * 512],
                        start=(kt == 0),
                        stop=(kt == KT2 - 1),
                    )
            nc.vector.tensor_copy(out=f1bc2[:], in_=f1p2[:])

        # ================= Phase D: layer-2 attention =====================
        with ExitStack() as dctx:
            pd_m = dctx.enter_context(tc.tile_pool(name="pd_m", bufs=3))
            pd_k = dctx.enter_context(tc.tile_pool(name="pd_k", bufs=3))
            pd_z = dctx.enter_context(tc.tile_pool(name="pd_z", bufs=2))
            pd_zl = dctx.enter_context(tc.tile_pool(name="pd_zl", bufs=2))
            pd_s = dctx.enter_context(tc.tile_pool(name="pd_s", bufs=2))
            pd_ps = dctx.enter_context(
                tc.tile_pool(name="pd_ps", bufs=1, space="PSUM")
            )

            o2T = pd_ps.tile([NCLASS + 1, ROWS], F32)

            # pre-pass: ACT-pair mask loads + f1+mask adds depend only on
            # f1bc2 (local) so they overlap the all-gather latency
            pd_ma = dctx.enter_context(tc.tile_pool(name="pd_ma", bufs=4))
            pd_za = dctx.enter_context(tc.tile_pool(name="pd_za", bufs=1))
            zt_pre = {}
            for jt2 in range(JT // 2):
                if L2_IS_DVE(jt2 * 2):
                    continue
                mta = pd_ma.tile([128, 2, ROWS], BF, tag="mta")
                dma.dma_start(
                    out=mta[:],
                    in_=maskT_d[jt2 * 256 : (jt2 + 1) * 256, :].rearrange(
                        "(t p) i -> p t i", p=128
                    ),
                )
                f1_bc2 = bass.AP(
                    tensor=f1bc2.tensor,
                    offset=f1bc2.offset,
                    ap=[f1bc2.ap[0], [0, 2], f1bc2.ap[1]],
                )
                zt = pd_za.tile([128, 2, ROWS], BF, tag=f"ztp{jt2}")
                nc.vector.tensor_tensor(
                    out=zt[:].rearrange("p t r -> p (t r)"),
                    in0=f1_bc2,
                    in1=mta[:].rearrange("p t r -> p (t r)"),
                    op=AluOp.add,
                )
                zt_pre[jt2] = zt

            for jt2 in range(JT // 2):
                if L2_IS_DVE(jt2 * 2):
                    mt = pd_m.tile([128, 2, ROWS], BF, tag="mt2")
                    dma.dma_start(
                        out=mt[:],
                        in_=maskT_d[jt2 * 256 : (jt2 + 1) * 256, :].rearrange(
                            "(t p) i -> p t i", p=128
                        ),
                    )
                    for t in range(2):
                        jt = jt2 * 2 + t
                        kt16 = pd_k.tile([128, ROWS], I16, tag="k16d")
                        nc.vector._custom_dve(
                            GAT_EXP,
                            out=kt16[:],
                            in0=f1bc2[:],
                            in1=mt[:, t, :],
                            s0=fstore2[:, jt : jt + 1],
                            s1=ALPHA,
                            imm2=BIAS,
                        )
                        for c in range(ROWS // 512):
                            nc.tensor.matmul(
                                o2T[:, c * 512 : (c + 1) * 512],
                                lhsT=h2_all[:, jt, :],
                                rhs=kt16[:, c * 512 : (c + 1) * 512].bitcast(BF),
                                start=(jt == 0),
                                stop=(jt == JT - 1),
                            )
                else:
                    zt = zt_pre[jt2]
                    zl = pd_zl.tile([128, 2, ROWS], BF, tag="zl2")
                    for t in range(2):
                        jt = jt2 * 2 + t
                        nc.scalar.activation(
                            out=zl[:, t, :],
                            in_=zt[:, t, :],
                            func=ActFn.Prelu,
                            bias=fstore2[:, jt : jt + 1],
                            scale=1.0,
                            alpha=ALPHA,
                        )
                    st = pd_s.tile([128, 2, ROWS], BF, tag="st2")
                    nc.scalar.activation(
                        out=st[:].rearrange("p t r -> p (t r)"),
                        in_=zl[:].rearrange("p t r -> p (t r)"),
                        func=ActFn.Exp,
                        bias=0.0,
                        scale=LN2_128,
                    )
                    for t in range(2):
                        jt = jt2 * 2 + t
                        for c in range(ROWS // 512):
                            nc.tensor.matmul(
                                o2T[:, c * 512 : (c + 1) * 512],
                                lhsT=h2_all[:, jt, :],
                                rhs=st[:, t, c * 512 : (c + 1) * 512],
                                start=(jt == 0),
                                stop=(jt == JT - 1),
                            )

            # epilogue: transpose back per i-tile, normalize
            pd_ep = dctx.enter_context(tc.tile_pool(name="pd_ep", bufs=1))
            o2sb = pd_ep.tile([NCLASS + 1, ROWS], F32)
            nc.vector.tensor_copy(out=o2sb[:], in_=o2T[:])
            ident = pd_ep.tile([128, 128], F32)
            from concourse.masks import make_identity

            make_identity(nc, ident[:])
            pd_tp = dctx.enter_context(
                tc.tile_pool(name="pd_tp", bufs=2, space="PSUM")
            )
            pd_r = dctx.enter_context(tc.tile_pool(name="pd_r", bufs=2))
            for it in range(IT):
                tp = pd_tp.tile([128, NCLASS + 1], F32, tag="tp")
                nc.tensor.transpose(
                    tp[:],
                    in_=o2sb[:, it * 128 : (it + 1) * 128],
                    identity=ident[0 : NCLASS + 1, 0 : NCLASS + 1],
                )
                r2 = pd_r.tile([128, 1], F32, tag="r2")
                nc.vector.reciprocal(out=r2[:], in_=tp[:, NCLASS : NCLASS + 1])
                nc.vector.tensor_scalar(
                    out_sb[:, it, :], tp[:, 0:NCLASS], r2[:], None, AluOp.mult
                )

        dma.dma_start(
            out=out_d[:, :].rearrange("(it p) c -> p it c", p=128),
            in_=out_sb[:],
        )

    _compile_with_single_act_table(nc)
    return nc


_NC_CACHE = {}


def _get_nc():
    if "nc" not in _NC_CACHE:
        _NC_CACHE["nc"] = build_nc()
    return _NC_CACHE["nc"]


def _host_prep(x, adj, Wh, ah, Wo, ao):
    x = np.asarray(x, np.float32)
    adj = np.asarray(adj, np.int32)
    Wh = np.asarray(Wh, np.float32)
    ah = np.asarray(ah, np.float32)
    Wo = np.asarray(Wo, np.float32)
    ao = np.asarray(ao, np.float32)

    xT = np.ascontiguousarray(x.T).astype(BF16)                    # [512, 8192]
    # additive mask in the 128*log2 domain, transposed: mask[j, i] masks
    # score of query-row i (own rows) vs source node j
    maskT = np.where(adj.T > 0, np.float32(0.0), np.float32(MASKV)).astype(BF16)

    wcat = np.concatenate(
        [np.concatenate([Wh[k] for k in range(NHEADS)], axis=1)]
        + [Wh[k] @ ah[k, NHID:, 0:1] * SCL for k in range(NHEADS)],
        axis=1,
    ).astype(BF16)                                                 # [512, 260]
    wa1 = np.stack(
        [Wh[k] @ ah[k, :NHID, 0] * SCL for k in range(NHEADS)], axis=1
    )
    wa1bc = np.broadcast_to(wa1[:, :, None], (NFEAT, NHEADS, 128)).astype(BF16)
    woext = np.concatenate([Wo, Wo @ ao[NCLASS:, 0:1] * SCL], axis=1).astype(BF16)
    woa1bc = np.broadcast_to(
        (Wo @ ao[:NCLASS, 0] * SCL)[:, None], (FCAT, 128)
    ).astype(BF16)

    in_maps = []
    for c in range(NCORES):
        r0 = c * ROWS
        in_maps.append(
            {
                "xT": xT,
                "xTown": np.ascontiguousarray(xT[:, r0 : r0 + ROWS]),
                "maskT": np.ascontiguousarray(maskT[:, r0 : r0 + ROWS]),
                "wcat": wcat,
                "wa1bc": np.ascontiguousarray(wa1bc),
                "woext": woext,
                "woa1bc": np.ascontiguousarray(woa1bc),
            }
        )
    return in_maps


def kernel(x, adj, Wh, ah, Wo, ao):
    nc = _get_nc()
    in_maps = _host_prep(x, adj, Wh, ah, Wo, ao)
    res = run_bass_kernel_spmd(
        nc,
        in_maps,
        core_ids=list(range(NCORES)),
        trace=bool(int(os.environ.get("GAT_TRACE", "0"))),
    )
    _NC_CACHE["last_results"] = res
    out = np.concatenate([res.results[c]["out"] for c in range(NCORES)], axis=0)
    return out.astype(np.float32)


if __name__ == "__main__":
    nc = build_nc()
    print("build+compile OK")


# revision 30
# speedup vs baseline: 1.1123x; 1.1123x over previous
"""GAT (2-layer, 4-head) Trainium2 Bass kernel, sharded across 8 NeuronCores.

Sharding: 1D row partition of the dense NxN attention; each core owns 1024
rows (queries). Full h = x @ W computed locally, row-block of masked softmax
attention and att @ h per layer, xcat all-gathered between layers.

v2 core trick — fused score+exp on the DVE via a custom 5-stage op:
    K = max(y, 0.2*y) + 16256,  y = (f1bc128 + f2_128[j]) + mask128
computed on scores pre-scaled by 128*log2e (folded into the weights on
host), written to an int16 tile. fp32->int16 write-conversion makes the
VALUE the bf16 BIT PATTERN: reinterpreting the int16 bytes as bf16 yields
2^lrelu(z)' (Schraudolph linear-interp exp2, |err|<=6%, cancels in softmax).
The att @ h matmul reads the tile via .bitcast(bf16) directly - one DVE
instruction per (head, j-tile), no ACT exp, no separate mask op.

Remaining units run an ACT path (gpsimd tensor-tensor add -> Prelu(bias=f2)
-> Exp(scale=ln2/128)) to balance DVE vs ACT vs GpSimd load. adj reaches the
device only as a host-precomputed bf16 additive mask {0, -229376}.
"""

import os
import sys
from contextlib import ExitStack

import numpy as np

sys.path.insert(0, "/opt/trn_rl_repo")

import ml_dtypes

import concourse.bass as bass
import concourse.tile as tile
from concourse import bacc, mybir
from concourse.bass_utils import run_bass_kernel_spmd

from concourse.dve_ops import (
    DveOp,
    OPS,
    CUSTOM_DVE_SPECS,
    _SUB_OPCODE_FOR_NAME,
    _CUSTOM_DVE_ROW_BASE,
)
from concourse.dve_spec import Spec, Src0, Src1, C0, C1, C2, maxx, lower, _has_src1
from concourse.dve_uop import DveOpSpec

BF16 = ml_dtypes.bfloat16
F32 = mybir.dt.float32
BF = mybir.dt.bfloat16
I16 = mybir.dt.int16

N, NFEAT, NHID, NCLASS, NHEADS, NCORES = 8192, 512, 64, 16, 4, 8
ROWS = N // NCORES          # 1024 rows per core
JT = N // 128               # 64 j-tiles
IT = ROWS // 128            # 8 i-tiles
KT1 = NFEAT // 128          # 4 k-tiles layer-1
FCAT = NHEADS * NHID        # 256
KT2 = FCAT // 128           # 2 k-tiles layer-2
ALPHA = 0.2
OUT_SLOPE = 0.01

SCL = 128.0 * 1.4426950408889634   # fold into f1/f2: scores in 128*log2 domain
MASKV = -229376.0                  # additive mask, bf16-exact (-1.75*2^17)
BIAS = 16256.0 - 7.364             # (127<<7) minus Schraudolph mean-centering
LN2_128 = float(np.log(2.0) / 128.0)

AluOp = mybir.AluOpType
ActFn = mybir.ActivationFunctionType

# --- path config (tunable): which (unit) runs the DVE fused path vs ACT ----
def L1_IS_DVE(k, jt):           # layer-1 (head, jt) on the fused DVE path?
    if k <= 1:
        return True
    if k == 2:
        return (jt // 2) % 8 in (0, 3, 6)
    return False
def L2_IS_DVE(jt):              # layer-2: pair-aligned, 2/3 on DVE
    return (jt // 2) % 3 != 2


def _register_gat_exp():
    """Fused masked-score -> Schraudolph-exp2 DVE op (int16 output)."""
    name = "GAT_EXP5B"
    if name in CUSTOM_DVE_SPECS:
        return next(op for op in OPS if op.name == name)

    def ref(in0, in1, s0, s1, imm2):
        y = in0.astype(np.float32) + np.float32(s0) + in1.astype(np.float32)
        l = np.maximum(y, np.float32(s1) * y)
        K = l + np.float32(imm2)
        return np.clip(np.rint(K), -32768, 32767).astype(np.int16)

    _y = (Src0 + C0) + Src1
    spec = Spec(body=maxx(_y, _y * C1) + C2, reference=ref)
    row = _CUSTOM_DVE_ROW_BASE + len(OPS)
    assert row < 0x20
    uops = lower(spec, ver="v3")
    sha = DveOpSpec(name=name, opcode=row, uops=uops, rd1_en=_has_src1(spec)).sha("v3")
    op = DveOp(name=name, spec=spec, subdim=False, uops_sha={"v3": sha})
    OPS.append(op)
    CUSTOM_DVE_SPECS[name] = spec
    _SUB_OPCODE_FOR_NAME[name] = row
    return op


GAT_EXP = _register_gat_exp()


def _compile_with_single_act_table(nc):
    """Restrict activations to two HW table sets: the Exp set (phases B/D)
    and the Reciprocal set (used once, in the phase-B epilogue)."""
    import concourse.bacc as bacc_mod

    orig = bacc_mod.get_activation_tables
    need = {
        mybir.ActivationFunctionType.Exp,
        mybir.ActivationFunctionType.Prelu,
        mybir.ActivationFunctionType.Copy,
        mybir.ActivationFunctionType.Identity,
    }
    need_r = {
        mybir.ActivationFunctionType.Reciprocal,
        mybir.ActivationFunctionType.Prelu,
        mybir.ActivationFunctionType.Copy,
    }

    def restricted(arch):
        tables = orig(arch)
        out = {}
        for k, v in tables.items():
            if need <= set(v):
                out[k] = v
                break
        for k, v in tables.items():
            if need_r <= set(v):
                out[k] = v
                break
        return out or tables

    bacc_mod.get_activation_tables = restricted
    try:
        nc.compile()
    finally:
        bacc_mod.get_activation_tables = orig


def build_nc():
    nc = bacc.Bacc(
        "TRN2", target_bir_lowering=False, debug=False, num_devices=NCORES
    )

    # ---- I/O -------------------------------------------------------------
    xT_d = nc.dram_tensor("xT", [NFEAT, N], BF, kind="ExternalInput")
    xTown_d = nc.dram_tensor("xTown", [NFEAT, ROWS], BF, kind="ExternalInput")
    maskT_d = nc.dram_tensor("maskT", [N, ROWS], BF, kind="ExternalInput")
    wcat_d = nc.dram_tensor("wcat", [NFEAT, FCAT + NHEADS], BF, kind="ExternalInput")
    wa1bc_d = nc.dram_tensor("wa1bc", [NFEAT, NHEADS, 128], BF, kind="ExternalInput")
    woext_d = nc.dram_tensor("woext", [FCAT, NCLASS + 1], BF, kind="ExternalInput")
    woa1bc_d = nc.dram_tensor("woa1bc", [FCAT, 128], BF, kind="ExternalInput")
    out_d = nc.dram_tensor("out", [ROWS, NCLASS], F32, kind="ExternalOutput")
    # layer-2 gather: own-rows h2 (16 cols bf16) + f2*SCL (1 col f32 as 2
    # bf16 cols) + pad, gathered across cores
    g_d = nc.dram_tensor("g_bounce", [ROWS, 20], BF, kind="Internal")
    gg_d = nc.dram_tensor(
        "gg_bounce", [N, 20], BF, kind="Internal", addr_space="Shared"
    )

    dma = nc.default_dma_engine

    with tile.TileContext(nc) as tc, ExitStack() as ctx:
        persist = ctx.enter_context(tc.tile_pool(name="persist", bufs=1))

        h_all = persist.tile([128, JT, NHEADS, NHID + 1], BF)
        fstore = persist.tile([128, JT, NHEADS], F32)      # f2 * SCL
        f1bc = persist.tile([128, NHEADS, ROWS], BF)       # f1 * SCL bcast
        xcT_sb = persist.tile([128, KT2, ROWS], BF)
        h2_all = persist.tile([128, JT, NCLASS + 1], BF)
        fstore2 = persist.tile([128, JT], F32)
        f1bc2 = persist.tile([128, ROWS], BF)
        out_sb = persist.tile([128, IT, NCLASS], F32)

        nc.gpsimd.memset(h_all[:, :, :, NHID : NHID + 1], 1.0)
        nc.gpsimd.memset(h2_all[:, :, NCLASS : NCLASS + 1], 1.0)

        # ================= Phase A: h + f1/f2 =============================
        with ExitStack() as actx:
            pa = actx.enter_context(tc.tile_pool(name="pa", bufs=1))
            pa_ps = actx.enter_context(
                tc.tile_pool(name="pa_ps", bufs=2, space="PSUM")
            )

            xT_sb = pa.tile([128, KT1, N], BF)
            for kt in range(KT1):
                dma.dma_start(
                    out=xT_sb[:, kt, :],
                    in_=xT_d[kt * 128 : (kt + 1) * 128, :],
                )
            xTown_sb = pa.tile([128, KT1, ROWS], BF)
            dma.dma_start(
                out=xTown_sb[:],
                in_=xTown_d[:, :].rearrange("(kt p) f -> p kt f", p=128),
            )
            wcat_sb = pa.tile([128, KT1, FCAT + NHEADS], BF)
            dma.dma_start(
                out=wcat_sb[:],
                in_=wcat_d[:, :].rearrange("(kt p) c -> p kt c", p=128),
            )
            wa1bc_sb = pa.tile([128, KT1, NHEADS, 128], BF)
            dma.dma_start(
                out=wa1bc_sb[:],
                in_=wa1bc_d[:, :, :].rearrange("(kt p) h m -> p kt h m", p=128),
            )

            for jt in range(JT):
                hp = pa_ps.tile([128, FCAT + NHEADS], F32, tag="hp")
                for kt in range(KT1):
                    nc.tensor.matmul(
                        hp[:],
                        lhsT=xT_sb[:, kt, jt * 128 : (jt + 1) * 128],
                        rhs=wcat_sb[:, kt, :],
                        start=(kt == 0),
                        stop=(kt == KT1 - 1),
                    )
                # h copy on ACT engine, f2 copy on DVE (small)
                nc.scalar.copy(
                    out=h_all[:, jt, :, 0:NHID],
                    in_=hp[:, 0:FCAT].rearrange("p (h d) -> p h d", h=NHEADS),
                )
                nc.vector.tensor_copy(
                    out=fstore[:, jt, :], in_=hp[:, FCAT : FCAT + NHEADS]
                )

            # f1 broadcast tiles [128, ROWS] per head
            for k in range(NHEADS):
                f1p = pa_ps.tile([128, ROWS], F32, tag="f1p")
                for kt in range(KT1):
                    for c in range(ROWS // 512):
                        nc.tensor.matmul(
                            f1p[:, c * 512 : (c + 1) * 512],
                            lhsT=wa1bc_sb[:, kt, k, :],
                            rhs=xTown_sb[:, kt, c * 512 : (c + 1) * 512],
                            start=(kt == 0),
                            stop=(kt == KT1 - 1),
                        )
                nc.vector.tensor_copy(out=f1bc[:, k, :], in_=f1p[:])

        # ================= Phase B: layer-1 attention =====================
        pe_sb = ctx.enter_context(tc.tile_pool(name="pe_sb", bufs=1))
        with ExitStack() as bctx:
            pb_m = bctx.enter_context(tc.tile_pool(name="pb_m", bufs=4))
            pb_k = bctx.enter_context(tc.tile_pool(name="pb_k", bufs=6))
            pb_z = bctx.enter_context(tc.tile_pool(name="pb_z", bufs=2))
            pb_zl = bctx.enter_context(tc.tile_pool(name="pb_zl", bufs=2))
            pb_s = bctx.enter_context(tc.tile_pool(name="pb_s", bufs=2))
            pb_ps = bctx.enter_context(
                tc.tile_pool(name="pb_ps", bufs=1, space="PSUM")
            )

            oT = [
                pb_ps.tile([NHID + 1, ROWS], F32, tag=f"oT{k}", name=f"oT{k}")
                for k in range(NHEADS)
            ]

            for jt2 in range(JT // 2):
                mt = pb_m.tile([128, 2, ROWS], BF, tag="mt")
                dma.dma_start(
                    out=mt[:],
                    in_=maskT_d[jt2 * 256 : (jt2 + 1) * 256, :].rearrange(
                        "(t p) i -> p t i", p=128
                    ),
                )
                for k in range(NHEADS):
                    if L1_IS_DVE(k, jt2 * 2):
                        # fused DVE path (per jt)
                        for t in range(2):
                            jt = jt2 * 2 + t
                            kt16 = pb_k.tile([128, ROWS], I16, tag=f"k16_{k}{t}")
                            nc.vector._custom_dve(
                                GAT_EXP,
                                out=kt16[:],
                                in0=f1bc[:, k, :],
                                in1=mt[:, t, :],
                                s0=fstore[:, jt, k : k + 1],
                                s1=ALPHA,
                                imm2=BIAS,
                            )
                            for c in range(ROWS // 512):
                                nc.tensor.matmul(
                                    oT[k][:, c * 512 : (c + 1) * 512],
                                    lhsT=h_all[:, jt, k, :],
                                    rhs=kt16[:, c * 512 : (c + 1) * 512].bitcast(BF),
                                    start=(jt == 0),
                                    stop=(jt == JT - 1),
                                )
                    else:
                        # ACT path, pair-batched TT on DVE
                        f1k = f1bc[:, k, :]
                        f1_bc2 = bass.AP(
                            tensor=f1k.tensor,
                            offset=f1k.offset,
                            ap=[f1k.ap[0], [0, 2], f1k.ap[1]],
                        )
                        zt = pb_z.tile([128, 2, ROWS], BF, tag=f"zt{k}")
                        nc.vector.tensor_tensor(
                            out=zt[:].rearrange("p t r -> p (t r)"),
                            in0=f1_bc2,
                            in1=mt[:].rearrange("p t r -> p (t r)"),
                            op=AluOp.add,
                        )
                        zl = pb_zl.tile([128, 2, ROWS], BF, tag=f"zl{k}")
                        for t in range(2):
                            jt = jt2 * 2 + t
                            nc.scalar.activation(
                                out=zl[:, t, :],
                                in_=zt[:, t, :],
                                func=ActFn.Prelu,
                                bias=fstore[:, jt, k : k + 1],
                                scale=1.0,
                                alpha=ALPHA,
                            )
                        st = pb_s.tile([128, 2, ROWS], BF, tag=f"st{k}")
                        nc.scalar.activation(
                            out=st[:].rearrange("p t r -> p (t r)"),
                            in_=zl[:].rearrange("p t r -> p (t r)"),
                            func=ActFn.Exp,
                            bias=0.0,
                            scale=LN2_128,
                        )
                        for t in range(2):
                            jt = jt2 * 2 + t
                            for c in range(ROWS // 512):
                                nc.tensor.matmul(
                                    oT[k][:, c * 512 : (c + 1) * 512],
                                    lhsT=h_all[:, jt, k, :],
                                    rhs=st[:, t, c * 512 : (c + 1) * 512],
                                    start=(jt == 0),
                                    stop=(jt == JT - 1),
                                )

            osb = [
                pe_sb.tile([NHID + 1, ROWS], F32, tag=f"osb{k}", name=f"osb{k}")
                for k in range(NHEADS)
            ]
            for k in range(NHEADS):
                if k % 2 == 0:
                    nc.vector.tensor_copy(out=osb[k][:], in_=oT[k][:])
                else:
                    nc.scalar.copy(out=osb[k][:], in_=oT[k][:])

        # weights for layer 2 (independent of the collective; load early)
        pw = ctx.enter_context(tc.tile_pool(name="pw", bufs=1))
        woext_sb = pw.tile([128, KT2, NCLASS + 1], BF)
        dma.dma_start(
            out=woext_sb[:],
            in_=woext_d[:, :].rearrange("(kt p) c -> p kt c", p=128),
        )
        woa1bc_sb = pw.tile([128, KT2, 128], BF)
        dma.dma_start(
            out=woa1bc_sb[:],
            in_=woa1bc_d[:, :].rearrange("(kt p) m -> p kt m", p=128),
        )

        # epilogue: normalize + out-lrelu + pack xcatT
        with ExitStack() as ectx:
            pe_ps = ectx.enter_context(
                tc.tile_pool(name="pe_ps", bufs=2, space="PSUM")
            )
            pe_u = ectx.enter_context(tc.tile_pool(name="pe_u", bufs=2))
            ones_sb = ectx.enter_context(tc.tile_pool(name="ones", bufs=1)).tile(
                [1, NHID], F32
            )
            nc.gpsimd.memset(ones_sb[:], 1.0)
            rrow = ectx.enter_context(tc.tile_pool(name="rrow", bufs=1))

            # gather the 4 denominator rows onto partitions 0-3, one DVE
            # reciprocal (cost = free size), scatter back to partition 0
            dd = rrow.tile([4, ROWS], F32)
            for k in range(NHEADS):
                dma.dma_start(out=dd[k : k + 1, :], in_=osb[k][NHID : NHID + 1, :])
            rr = rrow.tile([4, ROWS], F32)
            nc.vector.reciprocal(out=rr[:], in_=dd[:])
            rs4 = [rrow.tile([1, ROWS], F32, name=f"rs{k}") for k in range(NHEADS)]
            for k in range(NHEADS):
                dma.dma_start(out=rs4[k][:], in_=rr[k : k + 1, :])

            for k in range(NHEADS):
                rs = rs4[k]
                rbc = pe_ps.tile([NHID, ROWS], F32, tag="rbc")
                for c in range(ROWS // 512):
                    nc.tensor.matmul(
                        rbc[:, c * 512 : (c + 1) * 512],
                        lhsT=ones_sb[:],
                        rhs=rs[:, c * 512 : (c + 1) * 512],
                        start=True,
                        stop=True,
                    )
                u = pe_u.tile([NHID, ROWS], F32, tag="u")
                nc.vector.tensor_tensor(
                    out=u[:], in0=osb[k][0:NHID, :], in1=rbc[:], op=AluOp.mult
                )
                nc.vector.scalar_tensor_tensor(
                    out=xcT_sb[(k % 2) * NHID : (k % 2) * NHID + NHID, k // 2, :],
                    in0=u[:],
                    scalar=OUT_SLOPE,
                    in1=u[:],
                    op0=AluOp.mult,
                    op1=AluOp.max,
                )

        # ============ Phase C: local h2/f2 for own rows, small all-gather ==
        with ExitStack() as cctx:
            pc = cctx.enter_context(tc.tile_pool(name="pc", bufs=1))
            pc_ps = cctx.enter_context(
                tc.tile_pool(name="pc_ps", bufs=2, space="PSUM")
            )

            h2own = pc.tile([128, IT, NCLASS], BF)
            f2own = pc.tile([128, IT, 1], F32)
            for it in range(IT):
                h2p = pc_ps.tile([128, NCLASS + 1], F32, tag="h2p")
                for kt in range(KT2):
                    nc.tensor.matmul(
                        h2p[:],
                        lhsT=xcT_sb[:, kt, it * 128 : (it + 1) * 128],
                        rhs=woext_sb[:, kt, :],
                        start=(kt == 0),
                        stop=(kt == KT2 - 1),
                    )
                nc.vector.tensor_copy(
                    out=h2own[:, it, :], in_=h2p[:, 0:NCLASS]
                )
                nc.vector.tensor_copy(
                    out=f2own[:, it, :], in_=h2p[:, NCLASS : NCLASS + 1]
                )
            dma.dma_start(
                out=g_d[:, 0:NCLASS].rearrange("(it p) c -> p it c", p=128),
                in_=h2own[:],
            )
            dma.dma_start(
                out=g_d[:, NCLASS : NCLASS + 2]
                .bitcast(F32)
                .rearrange("(it p) c -> p it c", p=128),
                in_=f2own[:],
            )
            nc.gpsimd.collective_compute(
                "AllGather",
                AluOp.bypass,
                replica_groups=[list(range(NCORES))],
                ins=[g_d[:, :].opt()],
                outs=[gg_d[:, :].opt()],
            )
            dma.dma_start(
                out=h2_all[:, :, 0:NCLASS],
                in_=gg_d[:, 0:NCLASS].rearrange("(jt p) c -> p jt c", p=128),
            )
            dma.dma_start(
                out=fstore2[:, :],
                in_=gg_d[:, NCLASS : NCLASS + 2]
                .bitcast(F32)
                .rearrange("(jt p) c -> p (jt c)", p=128),
            )

            f1p2 = pc_ps.tile([128, ROWS], F32, tag="f1p2")
            for kt in range(KT2):
                for c in range(ROWS // 512):
                    nc.tensor.matmul(
                        f1p2[:, c * 512 : (c + 1) * 512],
                        lhsT=woa1bc_sb[:, kt, :],
                        rhs=xcT_sb[:, kt, c * 512 : (c + 1) * 512],
                        start=(kt == 0),
                        stop=(kt == KT2 - 1),
                    )
            nc.vector.tensor_copy(out=f1bc2[:], in_=f1p2[:])

        # ================= Phase D: layer-2 attention =====================
        with ExitStack() as dctx:
            pd_m = dctx.enter_context(tc.tile_pool(name="pd_m", bufs=3))
            pd_k = dctx.enter_context(tc.tile_pool(name="pd_k", bufs=3))
            pd_z = dctx.enter_context(tc.tile_pool(name="pd_z", bufs=2))
            pd_zl = dctx.enter_context(tc.tile_pool(name="pd_zl", bufs=2))
            pd_s = dctx.enter_context(tc.tile_pool(name="pd_s", bufs=2))
            pd_ps = dctx.enter_context(
                tc.tile_pool(name="pd_ps", bufs=1, space="PSUM")
            )

            o2T = pd_ps.tile([NCLASS + 1, ROWS], F32)

            # pre-pass: ACT-pair mask loads + f1+mask adds depend only on
            # f1bc2 (local) so they overlap the all-gather latency
            pd_ma = dctx.enter_context(tc.tile_pool(name="pd_ma", bufs=4))
            pd_za = dctx.enter_context(tc.tile_pool(name="pd_za", bufs=1))
            zt_pre = {}
            for jt2 in range(JT // 2):
                if L2_IS_DVE(jt2 * 2):
                    continue
                mta = pd_ma.tile([128, 2, ROWS], BF, tag="mta")
                dma.dma_start(
                    out=mta[:],
                    in_=maskT_d[jt2 * 256 : (jt2 + 1) * 256, :].rearrange(
                        "(t p) i -> p t i", p=128
                    ),
                )
                f1_bc2 = bass.AP(
                    tensor=f1bc2.tensor,
                    offset=f1bc2.offset,
                    ap=[f1bc2.ap[0], [0, 2], f1bc2.ap[1]],
                )
                zt = pd_za.tile([128, 2, ROWS], BF, tag=f"ztp{jt2}")
                nc.vector.tensor_tensor(
                    out=zt[:].rearrange("p t r -> p (t r)"),
                    in0=f1_bc2,
                    in1=mta[:].rearrange("p t r -> p (t r)"),
                    op=AluOp.add,
                )
                zt_pre[jt2] = zt

            for jt2 in range(JT // 2):
                if L2_IS_DVE(jt2 * 2):
                    mt = pd_m.tile([128, 2, ROWS], BF, tag="mt2")
                    dma.dma_start(
                        out=mt[:],
                        in_=maskT_d[jt2 * 256 : (jt2 + 1) * 256, :].rearrange(
                            "(t p) i -> p t i", p=128
                        ),
                    )
                    for t in range(2):
                        jt = jt2 * 2 + t
                        kt16 = pd_k.tile([128, ROWS], I16, tag="k16d")
                        nc.vector._custom_dve(
                            GAT_EXP,
                            out=kt16[:],
                            in0=f1bc2[:],
                            in1=mt[:, t, :],
                            s0=fstore2[:, jt : jt + 1],
                            s1=ALPHA,
                            imm2=BIAS,
                        )
                        for c in range(ROWS // 512):
                            nc.tensor.matmul(
                                o2T[:, c * 512 : (c + 1) * 512],
                                lhsT=h2_all[:, jt, :],
                                rhs=kt16[:, c * 512 : (c + 1) * 512].bitcast(BF),
                                start=(jt == 0),
                                stop=(jt == JT - 1),
                            )
                else:
                    zt = zt_pre[jt2]
                    zl = pd_zl.tile([128, 2, ROWS], BF, tag="zl2")
                    for t in range(2):
                        jt = jt2 * 2 + t
                        nc.scalar.activation(
                            out=zl[:, t, :],
                            in_=zt[:, t, :],
                            func=ActFn.Prelu,
                            bias=fstore2[:, jt : jt + 1],
                            scale=1.0,
                            alpha=ALPHA,
                        )
                    st = pd_s.tile([128, 2, ROWS], BF, tag="st2")
                    nc.scalar.activation(
                        out=st[:].rearrange("p t r -> p (t r)"),
                        in_=zl[:].rearrange("p t r -> p (t r)"),
                        func=ActFn.Exp,
                        bias=0.0,
                        scale=LN2_128,
                    )
                    for t in range(2):
                        jt = jt2 * 2 + t
                        for c in range(ROWS // 512):
                            nc.tensor.matmul(
                                o2T[:, c * 512 : (c + 1) * 512],
                                lhsT=h2_all[:, jt, :],
                                rhs=st[:, t, c * 512 : (c + 1) * 512],
                                start=(jt == 0),
                                stop=(jt == JT - 1),
                            )

            # epilogue: transpose back per i-tile, normalize
            pd_ep = dctx.enter_context(tc.tile_pool(name="pd_ep", bufs=1))
            o2sb = pd_ep.tile([NCLASS + 1, ROWS], F32)
            nc.vector.tensor_copy(out=o2sb[:], in_=o2T[:])
            ident = pd_ep.tile([128, 128], F32)
            from concourse.masks import make_identity

            make_identity(nc, ident[:])
            pd_tp = dctx.enter_context(
                tc.tile_pool(name="pd_tp", bufs=2, space="PSUM")
            )
            pd_r = dctx.enter_context(tc.tile_pool(name="pd_r", bufs=2))
            for it in range(IT):
                tp = pd_tp.tile([128, NCLASS + 1], F32, tag="tp")
                nc.tensor.transpose(
                    tp[:],
                    in_=o2sb[:, it * 128 : (it + 1) * 128],
                    identity=ident[0 : NCLASS + 1, 0 : NCLASS + 1],
                )
                r2 = pd_r.tile([128, 1], F32, tag="r2")
                nc.vector.reciprocal(out=r2[:], in_=tp[:, NCLASS : NCLASS + 1])
                nc.vector.tensor_scalar(
                    out_sb[:, it, :], tp[:, 0:NCLASS], r2[:], None, AluOp.mult
                )

        dma.dma_start(
            out=out_d[:, :].rearrange("(it p) c -> p it c", p=128),
            in_=out_sb[:],
        )

    _compile_with_single_act_table(nc)
    return nc


_NC_CACHE = {}


def _get_nc():
    if "nc" not in _NC_CACHE:
        _NC_CACHE["nc"] = build_nc()
    return _NC_CACHE["nc"]


def _host_prep(x, adj, Wh, ah, Wo, ao):
    x = np.asarray(x, np.float32)
    adj = np.asarray(adj, np.int32)
    Wh = np.asarray(Wh, np.float32)
    ah = np.asarray(ah, np.float32)
    Wo = np.asarray(Wo, np.float32)
    ao = np.asarray(ao, np.float32)

    xT = np.ascontiguousarray(x.T).astype(BF16)                    # [512, 8192]
    # additive mask in the 128*log2 domain, transposed: mask[j, i] masks
    # score of query-row i (own rows) vs source node j
    maskT = np.where(adj.T > 0, np.float32(0.0), np.float32(MASKV)).astype(BF16)

    wcat = np.concatenate(
        [np.concatenate([Wh[k] for k in range(NHEADS)], axis=1)]
        + [Wh[k] @ ah[k, NHID:, 0:1] * SCL for k in range(NHEADS)],
        axis=1,
    ).astype(BF16)                                                 # [512, 260]
    wa1 = np.stack(
        [Wh[k] @ ah[k, :NHID, 0] * SCL for k in range(NHEADS)], axis=1
    )
    wa1bc = np.broadcast_to(wa1[:, :, None], (NFEAT, NHEADS, 128)).astype(BF16)
    woext = np.concatenate([Wo, Wo @ ao[NCLASS:, 0:1] * SCL], axis=1).astype(BF16)
    woa1bc = np.broadcast_to(
        (Wo @ ao[:NCLASS, 0] * SCL)[:, None], (FCAT, 128)
    ).astype(BF16)

    in_maps = []
    for c in range(NCORES):
        r0 = c * ROWS
        in_maps.append(
            {
                "xT": xT,
                "xTown": np.ascontiguousarray(xT[:, r0 : r0 + ROWS]),
                "maskT": np.ascontiguousarray(maskT[:, r0 : r0 + ROWS]),
                "wcat": wcat,
                "wa1bc": np.ascontiguousarray(wa1bc),
                "woext": woext,
                "woa1bc": np.ascontiguousarray(woa1bc),
            }
        )
    return in_maps


def kernel(x, adj, Wh, ah, Wo, ao):
    nc = _get_nc()
    in_maps = _host_prep(x, adj, Wh, ah, Wo, ao)
    res = run_bass_kernel_spmd(
        nc,
        in_maps,
        core_ids=list(range(NCORES)),
        trace=bool(int(os.environ.get("GAT_TRACE", "0"))),
    )
    _NC_CACHE["last_results"] = res
    out = np.concatenate([res.results[c]["out"] for c in range(NCORES)], axis=0)
    return out.astype(np.float32)


if __name__ == "__main__":
    nc = build_nc()
    print("build+compile OK")


# revision 34
# speedup vs baseline: 1.1235x; 1.0101x over previous
"""GAT (2-layer, 4-head) Trainium2 Bass kernel, sharded across 8 NeuronCores.

Sharding: 1D row partition of the dense NxN attention; each core owns 1024
rows (queries). Full h = x @ W computed locally, row-block of masked softmax
attention and att @ h per layer, xcat all-gathered between layers.

v2 core trick — fused score+exp on the DVE via a custom 5-stage op:
    K = max(y, 0.2*y) + 16256,  y = (f1bc128 + f2_128[j]) + mask128
computed on scores pre-scaled by 128*log2e (folded into the weights on
host), written to an int16 tile. fp32->int16 write-conversion makes the
VALUE the bf16 BIT PATTERN: reinterpreting the int16 bytes as bf16 yields
2^lrelu(z)' (Schraudolph linear-interp exp2, |err|<=6%, cancels in softmax).
The att @ h matmul reads the tile via .bitcast(bf16) directly - one DVE
instruction per (head, j-tile), no ACT exp, no separate mask op.

Remaining units run an ACT path (gpsimd tensor-tensor add -> Prelu(bias=f2)
-> Exp(scale=ln2/128)) to balance DVE vs ACT vs GpSimd load. adj reaches the
device only as a host-precomputed bf16 additive mask {0, -229376}.
"""

import os
import sys
from contextlib import ExitStack

import numpy as np

sys.path.insert(0, "/opt/trn_rl_repo")

import ml_dtypes

import concourse.bass as bass
import concourse.tile as tile
from concourse import bacc, mybir
from concourse.bass_utils import run_bass_kernel_spmd

from concourse.dve_ops import (
    DveOp,
    OPS,
    CUSTOM_DVE_SPECS,
    _SUB_OPCODE_FOR_NAME,
    _CUSTOM_DVE_ROW_BASE,
)
from concourse.dve_spec import Spec, Src0, Src1, C0, C1, C2, maxx, lower, _has_src1
from concourse.dve_uop import DveOpSpec

BF16 = ml_dtypes.bfloat16
F32 = mybir.dt.float32
BF = mybir.dt.bfloat16
I16 = mybir.dt.int16

N, NFEAT, NHID, NCLASS, NHEADS, NCORES = 8192, 512, 64, 16, 4, 8
ROWS = N // NCORES          # 1024 rows per core
JT = N // 128               # 64 j-tiles
IT = ROWS // 128            # 8 i-tiles
KT1 = NFEAT // 128          # 4 k-tiles layer-1
FCAT = NHEADS * NHID        # 256
KT2 = FCAT // 128           # 2 k-tiles layer-2
ALPHA = 0.2
OUT_SLOPE = 0.01

SCL = 128.0 * 1.4426950408889634   # fold into f1/f2: scores in 128*log2 domain
MASKV = -229376.0                  # additive mask, bf16-exact (-1.75*2^17)
BIAS = 16256.0 - 7.364             # (127<<7) minus Schraudolph mean-centering
LN2_128 = float(np.log(2.0) / 128.0)

AluOp = mybir.AluOpType
ActFn = mybir.ActivationFunctionType

# --- path config (tunable): which (unit) runs the DVE fused path vs ACT ----
def L1_IS_DVE(k, jt):           # layer-1 (head, jt) on the fused DVE path?
    if k <= 1:
        return True
    if k == 2:
        return (jt // 2) % 8 in (0, 3, 6)
    return False
def L2_IS_DVE(jt):              # layer-2: pair-aligned, 2/3 on DVE
    return (jt // 2) % 3 != 2


def _register_gat_exp():
    """Fused masked-score -> Schraudolph-exp2 DVE op (int16 output)."""
    name = "GAT_EXP5B"
    if name in CUSTOM_DVE_SPECS:
        return next(op for op in OPS if op.name == name)

    def ref(in0, in1, s0, s1, imm2):
        y = in0.astype(np.float32) + np.float32(s0) + in1.astype(np.float32)
        l = np.maximum(y, np.float32(s1) * y)
        K = l + np.float32(imm2)
        return np.clip(np.rint(K), -32768, 32767).astype(np.int16)

    _y = (Src0 + C0) + Src1
    spec = Spec(body=maxx(_y, _y * C1) + C2, reference=ref)
    row = _CUSTOM_DVE_ROW_BASE + len(OPS)
    assert row < 0x20
    uops = lower(spec, ver="v3")
    sha = DveOpSpec(name=name, opcode=row, uops=uops, rd1_en=_has_src1(spec)).sha("v3")
    op = DveOp(name=name, spec=spec, subdim=False, uops_sha={"v3": sha})
    OPS.append(op)
    CUSTOM_DVE_SPECS[name] = spec
    _SUB_OPCODE_FOR_NAME[name] = row
    return op


GAT_EXP = _register_gat_exp()


def _compile_with_single_act_table(nc):
    """Restrict activations to two HW table sets: the Exp set (phases B/D)
    and the Reciprocal set (used once, in the phase-B epilogue)."""
    import concourse.bacc as bacc_mod

    orig = bacc_mod.get_activation_tables
    need = {
        mybir.ActivationFunctionType.Exp,
        mybir.ActivationFunctionType.Prelu,
        mybir.ActivationFunctionType.Copy,
        mybir.ActivationFunctionType.Identity,
    }
    need_r = {
        mybir.ActivationFunctionType.Reciprocal,
        mybir.ActivationFunctionType.Prelu,
        mybir.ActivationFunctionType.Copy,
    }

    def restricted(arch):
        tables = orig(arch)
        out = {}
        for k, v in tables.items():
            if need <= set(v):
                out[k] = v
                break
        for k, v in tables.items():
            if need_r <= set(v):
                out[k] = v
                break
        return out or tables

    bacc_mod.get_activation_tables = restricted
    try:
        nc.compile()
    finally:
        bacc_mod.get_activation_tables = orig


def build_nc():
    nc = bacc.Bacc(
        "TRN2", target_bir_lowering=False, debug=False, num_devices=NCORES
    )

    # ---- I/O -------------------------------------------------------------
    xT_d = nc.dram_tensor("xT", [NFEAT, N], BF, kind="ExternalInput")
    xTown_d = nc.dram_tensor("xTown", [NFEAT, ROWS], BF, kind="ExternalInput")
    maskT_d = nc.dram_tensor("maskT", [N, ROWS], BF, kind="ExternalInput")
    wcat_d = nc.dram_tensor("wcat", [NFEAT, FCAT + NHEADS], BF, kind="ExternalInput")
    wa1bc_d = nc.dram_tensor("wa1bc", [NFEAT, NHEADS, 128], BF, kind="ExternalInput")
    woext_d = nc.dram_tensor("woext", [FCAT, NCLASS + 1], BF, kind="ExternalInput")
    woa1bc_d = nc.dram_tensor("woa1bc", [FCAT, 128], BF, kind="ExternalInput")
    out_d = nc.dram_tensor("out", [ROWS, NCLASS], F32, kind="ExternalOutput")
    # layer-2 gather: own-rows h2 (16 cols bf16) + f2*SCL (1 col f32 as 2
    # bf16 cols) + pad, gathered across cores
    g_d = nc.dram_tensor("g_bounce", [ROWS, 20], BF, kind="Internal")
    gg_d = nc.dram_tensor(
        "gg_bounce", [N, 20], BF, kind="Internal", addr_space="Shared"
    )

    dma = nc.default_dma_engine

    with tile.TileContext(nc) as tc, ExitStack() as ctx:
        persist = ctx.enter_context(tc.tile_pool(name="persist", bufs=1))

        h_all = persist.tile([128, JT, NHEADS, NHID + 1], BF)
        fstore = persist.tile([128, JT, NHEADS], F32)      # f2 * SCL
        f1bc = persist.tile([128, NHEADS, ROWS], BF)       # f1 * SCL bcast
        xcT_sb = persist.tile([128, KT2, ROWS], BF)
        h2_all = persist.tile([128, JT, NCLASS + 1], BF)
        fstore2 = persist.tile([128, JT], F32)
        f1bc2 = persist.tile([128, ROWS], BF)
        out_sb = persist.tile([128, IT, NCLASS], F32)

        nc.gpsimd.memset(h_all[:, :, :, NHID : NHID + 1], 1.0)
        nc.gpsimd.memset(h2_all[:, :, NCLASS : NCLASS + 1], 1.0)

        # ================= Phase A: h + f1/f2 =============================
        with ExitStack() as actx:
            pa = actx.enter_context(tc.tile_pool(name="pa", bufs=1))
            pa_ps = actx.enter_context(
                tc.tile_pool(name="pa_ps", bufs=2, space="PSUM")
            )

            xT_sb = pa.tile([128, KT1, N], BF)
            for kt in range(KT1):
                dma.dma_start(
                    out=xT_sb[:, kt, :],
                    in_=xT_d[kt * 128 : (kt + 1) * 128, :],
                )
            xTown_sb = pa.tile([128, KT1, ROWS], BF)
            dma.dma_start(
                out=xTown_sb[:],
                in_=xTown_d[:, :].rearrange("(kt p) f -> p kt f", p=128),
            )
            wcat_sb = pa.tile([128, KT1, FCAT + NHEADS], BF)
            dma.dma_start(
                out=wcat_sb[:],
                in_=wcat_d[:, :].rearrange("(kt p) c -> p kt c", p=128),
            )
            wa1bc_sb = pa.tile([128, KT1, NHEADS, 128], BF)
            dma.dma_start(
                out=wa1bc_sb[:],
                in_=wa1bc_d[:, :, :].rearrange("(kt p) h m -> p kt h m", p=128),
            )

            for jt in range(JT):
                hp = pa_ps.tile([128, FCAT + NHEADS], F32, tag="hp")
                for kt in range(KT1):
                    nc.tensor.matmul(
                        hp[:],
                        lhsT=xT_sb[:, kt, jt * 128 : (jt + 1) * 128],
                        rhs=wcat_sb[:, kt, :],
                        start=(kt == 0),
                        stop=(kt == KT1 - 1),
                    )
                # h copy on ACT engine, f2 copy on DVE (small)
                nc.scalar.copy(
                    out=h_all[:, jt, :, 0:NHID],
                    in_=hp[:, 0:FCAT].rearrange("p (h d) -> p h d", h=NHEADS),
                )
                nc.vector.tensor_copy(
                    out=fstore[:, jt, :], in_=hp[:, FCAT : FCAT + NHEADS]
                )

            # f1 broadcast tiles [128, ROWS] per head
            for k in range(NHEADS):
                f1p = pa_ps.tile([128, ROWS], F32, tag="f1p")
                for kt in range(KT1):
                    for c in range(ROWS // 512):
                        nc.tensor.matmul(
                            f1p[:, c * 512 : (c + 1) * 512],
                            lhsT=wa1bc_sb[:, kt, k, :],
                            rhs=xTown_sb[:, kt, c * 512 : (c + 1) * 512],
                            start=(kt == 0),
                            stop=(kt == KT1 - 1),
                        )
                nc.vector.tensor_copy(out=f1bc[:, k, :], in_=f1p[:])

        # ================= Phase B: layer-1 attention =====================
        pe_sb = ctx.enter_context(tc.tile_pool(name="pe_sb", bufs=1))
        with ExitStack() as bctx:
            pb_m = bctx.enter_context(tc.tile_pool(name="pb_m", bufs=4))
            pb_k = bctx.enter_context(tc.tile_pool(name="pb_k", bufs=6))
            pb_z = bctx.enter_context(tc.tile_pool(name="pb_z", bufs=2))
            pb_zl = bctx.enter_context(tc.tile_pool(name="pb_zl", bufs=2))
            pb_s = bctx.enter_context(tc.tile_pool(name="pb_s", bufs=2))
            pb_ps = bctx.enter_context(
                tc.tile_pool(name="pb_ps", bufs=1, space="PSUM")
            )

            oT = [
                pb_ps.tile([NHID + 1, ROWS], F32, tag=f"oT{k}", name=f"oT{k}")
                for k in range(NHEADS)
            ]

            for jt2 in range(JT // 2):
                mt = pb_m.tile([128, 2, ROWS], BF, tag="mt")
                dma.dma_start(
                    out=mt[:],
                    in_=maskT_d[jt2 * 256 : (jt2 + 1) * 256, :].rearrange(
                        "(t p) i -> p t i", p=128
                    ),
                )
                for k in range(NHEADS):
                    if L1_IS_DVE(k, jt2 * 2):
                        # fused DVE path (per jt)
                        for t in range(2):
                            jt = jt2 * 2 + t
                            kt16 = pb_k.tile([128, ROWS], I16, tag=f"k16_{k}{t}")
                            nc.vector._custom_dve(
                                GAT_EXP,
                                out=kt16[:],
                                in0=f1bc[:, k, :],
                                in1=mt[:, t, :],
                                s0=fstore[:, jt, k : k + 1],
                                s1=ALPHA,
                                imm2=BIAS,
                            )
                            for c in range(ROWS // 512):
                                nc.tensor.matmul(
                                    oT[k][:, c * 512 : (c + 1) * 512],
                                    lhsT=h_all[:, jt, k, :],
                                    rhs=kt16[:, c * 512 : (c + 1) * 512].bitcast(BF),
                                    start=(jt == 0),
                                    stop=(jt == JT - 1),
                                )
                    else:
                        # ACT path, pair-batched TT on DVE
                        f1k = f1bc[:, k, :]
                        f1_bc2 = bass.AP(
                            tensor=f1k.tensor,
                            offset=f1k.offset,
                            ap=[f1k.ap[0], [0, 2], f1k.ap[1]],
                        )
                        zt = pb_z.tile([128, 2, ROWS], BF, tag=f"zt{k}")
                        nc.vector.tensor_tensor(
                            out=zt[:].rearrange("p t r -> p (t r)"),
                            in0=f1_bc2,
                            in1=mt[:].rearrange("p t r -> p (t r)"),
                            op=AluOp.add,
                        )
                        zl = pb_zl.tile([128, 2, ROWS], BF, tag=f"zl{k}")
                        for t in range(2):
                            jt = jt2 * 2 + t
                            nc.scalar.activation(
                                out=zl[:, t, :],
                                in_=zt[:, t, :],
                                func=ActFn.Prelu,
                                bias=fstore[:, jt, k : k + 1],
                                scale=1.0,
                                alpha=ALPHA,
                            )
                        st = pb_s.tile([128, 2, ROWS], BF, tag=f"st{k}")
                        nc.scalar.activation(
                            out=st[:].rearrange("p t r -> p (t r)"),
                            in_=zl[:].rearrange("p t r -> p (t r)"),
                            func=ActFn.Exp,
                            bias=0.0,
                            scale=LN2_128,
                        )
                        for t in range(2):
                            jt = jt2 * 2 + t
                            for c in range(ROWS // 512):
                                nc.tensor.matmul(
                                    oT[k][:, c * 512 : (c + 1) * 512],
                                    lhsT=h_all[:, jt, k, :],
                                    rhs=st[:, t, c * 512 : (c + 1) * 512],
                                    start=(jt == 0),
                                    stop=(jt == JT - 1),
                                )

            osb = [
                pe_sb.tile([NHID + 1, ROWS], F32, tag=f"osb{k}", name=f"osb{k}")
                for k in range(NHEADS)
            ]
            for k in range(NHEADS):
                if k % 2 == 0:
                    nc.vector.tensor_copy(out=osb[k][:], in_=oT[k][:])
                else:
                    nc.scalar.copy(out=osb[k][:], in_=oT[k][:])

        # weights for layer 2 (independent of the collective; load early)
        pw = ctx.enter_context(tc.tile_pool(name="pw", bufs=1))
        woext_sb = pw.tile([128, KT2, NCLASS + 1], BF)
        dma.dma_start(
            out=woext_sb[:],
            in_=woext_d[:, :].rearrange("(kt p) c -> p kt c", p=128),
        )
        woa1bc_sb = pw.tile([128, KT2, 128], BF)
        dma.dma_start(
            out=woa1bc_sb[:],
            in_=woa1bc_d[:, :].rearrange("(kt p) m -> p kt m", p=128),
        )

        # epilogue: normalize + out-lrelu + pack xcatT
        with ExitStack() as ectx:
            pe_ps = ectx.enter_context(
                tc.tile_pool(name="pe_ps", bufs=2, space="PSUM")
            )
            pe_u = ectx.enter_context(tc.tile_pool(name="pe_u", bufs=2))
            ones_sb = ectx.enter_context(tc.tile_pool(name="ones", bufs=1)).tile(
                [1, NHID], F32
            )
            nc.gpsimd.memset(ones_sb[:], 1.0)
            rrow = ectx.enter_context(tc.tile_pool(name="rrow", bufs=1))

            # gather the 4 denominator rows onto partitions 0-3, one DVE
            # reciprocal (cost = free size), scatter back to partition 0
            dd = rrow.tile([4, ROWS], F32)
            for k in range(NHEADS):
                dma.dma_start(out=dd[k : k + 1, :], in_=osb[k][NHID : NHID + 1, :])
            rr = rrow.tile([4, ROWS], F32)
            nc.vector.reciprocal(out=rr[:], in_=dd[:])
            rs4 = [rrow.tile([1, ROWS], F32, name=f"rs{k}") for k in range(NHEADS)]
            for k in range(NHEADS):
                dma.dma_start(out=rs4[k][:], in_=rr[k : k + 1, :])

            for k in range(NHEADS):
                rs = rs4[k]
                rbc = pe_ps.tile([NHID, ROWS], F32, tag="rbc")
                for c in range(ROWS // 512):
                    nc.tensor.matmul(
                        rbc[:, c * 512 : (c + 1) * 512],
                        lhsT=ones_sb[:],
                        rhs=rs[:, c * 512 : (c + 1) * 512],
                        start=True,
                        stop=True,
                    )
                u = pe_u.tile([NHID, ROWS], F32, tag="u")
                nc.vector.tensor_tensor(
                    out=u[:], in0=osb[k][0:NHID, :], in1=rbc[:], op=AluOp.mult
                )
                nc.vector.scalar_tensor_tensor(
                    out=xcT_sb[(k % 2) * NHID : (k % 2) * NHID + NHID, k // 2, :],
                    in0=u[:],
                    scalar=OUT_SLOPE,
                    in1=u[:],
                    op0=AluOp.mult,
                    op1=AluOp.max,
                )

        # ============ Phase C: local h2/f2 for own rows, small all-gather ==
        with ExitStack() as cctx:
            pc = cctx.enter_context(tc.tile_pool(name="pc", bufs=1))
            pc_ps = cctx.enter_context(
                tc.tile_pool(name="pc_ps", bufs=2, space="PSUM")
            )

            h2own = pc.tile([128, IT, NCLASS], BF)
            f2own = pc.tile([128, IT, 1], F32)
            for it in range(IT):
                h2p = pc_ps.tile([128, NCLASS + 1], F32, tag="h2p")
                for kt in range(KT2):
                    nc.tensor.matmul(
                        h2p[:],
                        lhsT=xcT_sb[:, kt, it * 128 : (it + 1) * 128],
                        rhs=woext_sb[:, kt, :],
                        start=(kt == 0),
                        stop=(kt == KT2 - 1),
                    )
                nc.vector.tensor_copy(
                    out=h2own[:, it, :], in_=h2p[:, 0:NCLASS]
                )
                nc.vector.tensor_copy(
                    out=f2own[:, it, :], in_=h2p[:, NCLASS : NCLASS + 1]
                )
            dma.dma_start(
                out=g_d[:, 0:NCLASS].rearrange("(it p) c -> p it c", p=128),
                in_=h2own[:],
            )
            dma.dma_start(
                out=g_d[:, NCLASS : NCLASS + 2]
                .bitcast(F32)
                .rearrange("(it p) c -> p it c", p=128),
                in_=f2own[:],
            )
            nc.gpsimd.collective_compute(
                "AllGather",
                AluOp.bypass,
                replica_groups=[list(range(NCORES))],
                ins=[g_d[:, :].opt()],
                outs=[gg_d[:, :].opt()],
            )
            dma.dma_start(
                out=h2_all[:, :, 0:NCLASS],
                in_=gg_d[:, 0:NCLASS].rearrange("(jt p) c -> p jt c", p=128),
            )
            dma.dma_start(
                out=fstore2[:, :],
                in_=gg_d[:, NCLASS : NCLASS + 2]
                .bitcast(F32)
                .rearrange("(jt p) c -> p (jt c)", p=128),
            )

            f1p2 = pc_ps.tile([128, ROWS], F32, tag="f1p2")
            for kt in range(KT2):
                for c in range(ROWS // 512):
                    nc.tensor.matmul(
                        f1p2[:, c * 512 : (c + 1) * 512],
                        lhsT=woa1bc_sb[:, kt, :],
                        rhs=xcT_sb[:, kt, c * 512 : (c + 1) * 512],
                        start=(kt == 0),
                        stop=(kt == KT2 - 1),
                    )
            nc.vector.tensor_copy(out=f1bc2[:], in_=f1p2[:])

        # ================= Phase D: layer-2 attention =====================
        with ExitStack() as dctx:
            pd_m = dctx.enter_context(tc.tile_pool(name="pd_m", bufs=3))
            pd_k = dctx.enter_context(tc.tile_pool(name="pd_k", bufs=3))
            pd_z = dctx.enter_context(tc.tile_pool(name="pd_z", bufs=2))
            pd_zl = dctx.enter_context(tc.tile_pool(name="pd_zl", bufs=2))
            pd_s = dctx.enter_context(tc.tile_pool(name="pd_s", bufs=2))
            pd_ps = dctx.enter_context(
                tc.tile_pool(name="pd_ps", bufs=1, space="PSUM")
            )

            o2T = pd_ps.tile([NCLASS + 1, ROWS], F32)

            # pre-pass: ACT-pair mask loads + f1+mask adds depend only on
            # f1bc2 (local) so they overlap the all-gather latency
            pd_ma = dctx.enter_context(tc.tile_pool(name="pd_ma", bufs=4))
            pd_za = dctx.enter_context(tc.tile_pool(name="pd_za", bufs=1))
            zt_pre = {}
            for jt2 in range(JT // 2):
                if L2_IS_DVE(jt2 * 2):
                    continue
                mta = pd_ma.tile([128, 2, ROWS], BF, tag="mta")
                dma.dma_start(
                    out=mta[:],
                    in_=maskT_d[jt2 * 256 : (jt2 + 1) * 256, :].rearrange(
                        "(t p) i -> p t i", p=128
                    ),
                )
                f1_bc2 = bass.AP(
                    tensor=f1bc2.tensor,
                    offset=f1bc2.offset,
                    ap=[f1bc2.ap[0], [0, 2], f1bc2.ap[1]],
                )
                zt = pd_za.tile([128, 2, ROWS], BF, tag=f"ztp{jt2}")
                nc.vector.tensor_tensor(
                    out=zt[:].rearrange("p t r -> p (t r)"),
                    in0=f1_bc2,
                    in1=mta[:].rearrange("p t r -> p (t r)"),
                    op=AluOp.add,
                )
                zt_pre[jt2] = zt

            for jt2 in range(JT // 2):
                if L2_IS_DVE(jt2 * 2):
                    mt = pd_m.tile([128, 2, ROWS], BF, tag="mt2")
                    dma.dma_start(
                        out=mt[:],
                        in_=maskT_d[jt2 * 256 : (jt2 + 1) * 256, :].rearrange(
                            "(t p) i -> p t i", p=128
                        ),
                    )
                    for t in range(2):
                        jt = jt2 * 2 + t
                        kt16 = pd_k.tile([128, ROWS], I16, tag="k16d")
                        nc.vector._custom_dve(
                            GAT_EXP,
                            out=kt16[:],
                            in0=f1bc2[:],
                            in1=mt[:, t, :],
                            s0=fstore2[:, jt : jt + 1],
                            s1=ALPHA,
                            imm2=BIAS,
                        )
                        for c in range(ROWS // 512):
                            nc.tensor.matmul(
                                o2T[:, c * 512 : (c + 1) * 512],
                                lhsT=h2_all[:, jt, :],
                                rhs=kt16[:, c * 512 : (c + 1) * 512].bitcast(BF),
                                start=(jt == 0),
                                stop=(jt == JT - 1),
                            )
                else:
                    zt = zt_pre[jt2]
                    zl = pd_zl.tile([128, 2, ROWS], BF, tag="zl2")
                    for t in range(2):
                        jt = jt2 * 2 + t
                        nc.scalar.activation(
                            out=zl[:, t, :],
                            in_=zt[:, t, :],
                            func=ActFn.Prelu,
                            bias=fstore2[:, jt : jt + 1],
                            scale=1.0,
                            alpha=ALPHA,
                        )
                    st = pd_s.tile([128, 2, ROWS], BF, tag="st2")
                    nc.scalar.activation(
                        out=st[:].rearrange("p t r -> p (t r)"),
                        in_=zl[:].rearrange("p t r -> p (t r)"),
                        func=ActFn.Exp,
                        bias=0.0,
                        scale=LN2_128,
                    )
                    for t in range(2):
                        jt = jt2 * 2 + t
                        for c in range(ROWS // 512):
                            nc.tensor.matmul(
                                o2T[:, c * 512 : (c + 1) * 512],
                                lhsT=h2_all[:, jt, :],
                                rhs=st[:, t, c * 512 : (c + 1) * 512],
                                start=(jt == 0),
                                stop=(jt == JT - 1),
                            )

            # epilogue: transpose back per i-tile, normalize
            pd_ep = dctx.enter_context(tc.tile_pool(name="pd_ep", bufs=1))
            o2sb = pd_ep.tile([NCLASS + 1, ROWS], F32)
            nc.vector.tensor_copy(out=o2sb[:], in_=o2T[:])
            ident = pd_ep.tile([128, 128], F32)
            from concourse.masks import make_identity

            make_identity(nc, ident[:])
            pd_tp = dctx.enter_context(
                tc.tile_pool(name="pd_tp", bufs=2, space="PSUM")
            )
            pd_r = dctx.enter_context(tc.tile_pool(name="pd_r", bufs=2))
            for it in range(IT):
                tp = pd_tp.tile([128, NCLASS + 1], F32, tag="tp")
                nc.tensor.transpose(
                    tp[:],
                    in_=o2sb[:, it * 128 : (it + 1) * 128],
                    identity=ident[0 : NCLASS + 1, 0 : NCLASS + 1],
                )
                r2 = pd_r.tile([128, 1], F32, tag="r2")
                nc.vector.reciprocal(out=r2[:], in_=tp[:, NCLASS : NCLASS + 1])
                nc.vector.tensor_scalar(
                    out_sb[:, it, :], tp[:, 0:NCLASS], r2[:], None, AluOp.mult
                )

        dma.dma_start(
            out=out_d[:, :].rearrange("(it p) c -> p it c", p=128),
            in_=out_sb[:],
        )

    _compile_with_single_act_table(nc)
    return nc


_NC_CACHE = {}


def _get_nc():
    if "nc" not in _NC_CACHE:
        _NC_CACHE["nc"] = build_nc()
    return _NC_CACHE["nc"]


def _host_prep(x, adj, Wh, ah, Wo, ao):
    x = np.asarray(x, np.float32)
    adj = np.asarray(adj, np.int32)
    Wh = np.asarray(Wh, np.float32)
    ah = np.asarray(ah, np.float32)
    Wo = np.asarray(Wo, np.float32)
    ao = np.asarray(ao, np.float32)

    xT = np.ascontiguousarray(x.T).astype(BF16)                    # [512, 8192]
    # additive mask in the 128*log2 domain, transposed: mask[j, i] masks
    # score of query-row i (own rows) vs source node j
    maskT = np.where(adj.T > 0, np.float32(0.0), np.float32(MASKV)).astype(BF16)

    wcat = np.concatenate(
        [np.concatenate([Wh[k] for k in range(NHEADS)], axis=1)]
        + [Wh[k] @ ah[k, NHID:, 0:1] * SCL for k in range(NHEADS)],
        axis=1,
    ).astype(BF16)                                                 # [512, 260]
    wa1 = np.stack(
        [Wh[k] @ ah[k, :NHID, 0] * SCL for k in range(NHEADS)], axis=1
    )
    wa1bc = np.broadcast_to(wa1[:, :, None], (NFEAT, NHEADS, 128)).astype(BF16)
    woext = np.concatenate([Wo, Wo @ ao[NCLASS:, 0:1] * SCL], axis=1).astype(BF16)
    woa1bc = np.broadcast_to(
        (Wo @ ao[:NCLASS, 0] * SCL)[:, None], (FCAT, 128)
    ).astype(BF16)

    in_maps = []
    for c in range(NCORES):
        r0 = c * ROWS
        in_maps.append(
            {
                "xT": xT,
                "xTown": np.ascontiguousarray(xT[:, r0 : r0 + ROWS]),
                "maskT": np.ascontiguousarray(maskT[:, r0 : r0 + ROWS]),
                "wcat": wcat,
                "wa1bc": np.ascontiguousarray(wa1bc),
                "woext": woext,
                "woa1bc": np.ascontiguousarray(woa1bc),
            }
        )
    return in_maps


def kernel(x, adj, Wh, ah, Wo, ao):
    nc = _get_nc()
    in_maps = _host_prep(x, adj, Wh, ah, Wo, ao)
    res = run_bass_kernel_spmd(
        nc,
        in_maps,
        core_ids=list(range(NCORES)),
        trace=bool(int(os.environ.get("GAT_TRACE", "0"))),
    )
    _NC_CACHE["last_results"] = res
    out = np.concatenate([res.results[c]["out"] for c in range(NCORES)], axis=0)
    return out.astype(np.float32)


if __name__ == "__main__":
    nc = build_nc()
    print("build+compile OK")
